# revision 5
# baseline (speedup 1.0000x reference)
"""GAT-layer kernel for Trainium2 (8 NeuronCores, SPMD data-parallel over batch).

Math per batch sample b (one sample per core):
    ft     = features_b @ W                      # [N, D]
    scores = ft @ ft^T + bias                    # [N, N]  (N == D)
    out_b  = softmax(scores, axis=-1) @ ft       # [N, D]

Implementation notes:
  - Projection matmuls run in float32r (TF32-class, 1 cyc/row) producing
    both layouts: ftT [d, n] (fp8 operand store for the Gram matmuls) and
    ft [n, d] (float32r store, the value/moving side of PV -> sets output
    precision ~1e-4).
  - Per 128-row query block: bias is pre-added into PSUM via a K=1
    broadcast matmul, Gram accumulates over 16 contraction tiles, then
    reduce_max(negate) -> ACT exp (PSUM read, fused row-sum) -> PE
    transpose of E -> PV in float32r -> scale by 1/rowsum -> DMA out.
  - The q-loop is software-pipelined by one iteration so the PE stream
    (Gram_q | transpose_{q-1}, PV_{q-1}) never waits on softmax.
"""

import sys

for _p in ("/opt/trn_rl_repo", "/root/.axon_site/_ro/trn_rl_repo"):
    if _p not in sys.path:
        sys.path.insert(0, _p)

import numpy as np

import concourse.bass as bass
import concourse.mybir as mybir
import concourse.tile as tile
from concourse import bacc
from concourse.bass_utils import run_bass_kernel_spmd
from concourse.masks import make_identity

B, N, F, D = 8, 2048, 128, 2048
P = 128
NT = N // P   # 16 row blocks
KT = D // P   # 16 contraction tiles
NCH = D // 512  # 4 psum chunks of 512

f32 = mybir.dt.float32
f32r = mybir.dt.float32r
bf16 = mybir.dt.bfloat16
fp8 = mybir.dt.float8e4

_built = {}


def _build(reps=1):
    nc = bacc.Bacc()
    feat_d = nc.dram_tensor("features", [N, F], f32r, kind="ExternalInput")
    w_d = nc.dram_tensor("attn_weights", [F, D], f32r, kind="ExternalInput")
    bias_d = nc.dram_tensor("attn_bias", [1, D], f32, kind="ExternalInput")
    out_d = nc.dram_tensor("out", [N, D], f32, kind="ExternalOutput")

    with tile.TileContext(nc) as tc:
      for _rep in range(reps):
        with tc.tile_pool(name="persist", bufs=1) as persist:
            # big resident stores
            ft_all = persist.tile([P, NT * D], f32r)    # ft row-block nt at cols [nt*D, (nt+1)*D)
            ftT_all = persist.tile([P, KT * N], fp8)    # ftT d-block dt at cols [dt*N, (dt+1)*N)
            id_f32r = persist.tile([P, P], f32r)
            id_bf16 = persist.tile([P, P], bf16)
            id_f32 = persist.tile([P, P], f32)
            make_identity(nc, id_f32)
            make_identity(nc, id_bf16)
            nc.vector.tensor_copy(id_f32r, id_f32)
            ones_fp8 = persist.tile([1, P], fp8)
            nc.vector.memset(ones_fp8, 1.0)
            bias_fp8 = persist.tile([1, D], fp8)

            # ---------------- phase 0/1: load + projection ----------------
            with (
                tc.tile_pool(name="proj", bufs=1) as proj,
                tc.tile_pool(name="ldtiles", bufs=3) as ldtiles,
                tc.tile_pool(name="proj_ps", bufs=4, space="PSUM") as proj_ps,
            ):
                w_sb = proj.tile([F, D], f32r)
                nc.sync.dma_start(out=w_sb, in_=w_d.ap())
                bias_f32 = proj.tile([1, D], f32)
                nc.sync.dma_start(out=bias_f32, in_=bias_d.ap())
                nc.vector.tensor_copy(bias_fp8, bias_f32)

                # featT [f, n] from feat [n, f] via PE transposes
                featT = proj.tile([F, N], f32r)
                for nt in range(NT):
                    ftile = ldtiles.tile([P, F], f32r, tag="ftile")
                    nc.sync.dma_start(out=ftile, in_=feat_d.ap()[nt * P:(nt + 1) * P, :])
                    tp = proj_ps.tile([P, P], f32r, tag="tp")
                    nc.tensor.transpose(tp, ftile, id_f32r)
                    nc.vector.tensor_copy(featT[:, nt * P:(nt + 1) * P], tp)

                # ftT (d-partition layout, fp8) : out[dt, n] = W[:, dt].T @ featT
                for dt in range(KT):
                    lhsT = w_sb[:, dt * P:(dt + 1) * P]
                    for c in range(NCH):
                        pp = proj_ps.tile([P, 512], f32, tag="pp")
                        nc.tensor.matmul(pp, lhsT, featT[:, c * 512:(c + 1) * 512],
                                         start=True, stop=True)
                        nc.scalar.activation(
                            ftT_all[:, dt * N + c * 512: dt * N + (c + 1) * 512],
                            pp, mybir.ActivationFunctionType.Copy)

                # ft (n-partition layout, f32r) : out[nt, d] = featT[:, nt].T @ W
                for nt in range(NT):
                    lhsT = featT[:, nt * P:(nt + 1) * P]
                    for c in range(NCH):
                        pp = proj_ps.tile([P, 512], f32, tag="pp")
                        nc.tensor.matmul(pp, lhsT, w_sb[:, c * 512:(c + 1) * 512],
                                         start=True, stop=True)
                        nc.vector.tensor_copy(
                            ft_all[:, nt * D + c * 512: nt * D + (c + 1) * 512], pp)

            # ---------------- phase 2: attention, pipelined by 1 ----------------
            with (
                tc.tile_pool(name="attn", bufs=2) as attn,
                tc.tile_pool(name="et_pool", bufs=1) as et_pool,
                tc.tile_pool(name="stats", bufs=2) as stats,
                tc.tile_pool(name="g_ps", bufs=1, space="PSUM") as g_ps,
                tc.tile_pool(name="pv_ps", bufs=1, space="PSUM") as pv_ps,
            ):
                Es = [None] * NT
                recips = [None] * NT
                for it in range(NT + 1):
                    if it < NT:
                        q = it
                        # Gram + bias for query block q
                        G = g_ps.tile([P, D], f32, tag="G")
                        for c in range(NCH):
                            nc.tensor.matmul(G[:, c * 512:(c + 1) * 512], ones_fp8,
                                             bias_fp8[:, c * 512:(c + 1) * 512],
                                             start=True, stop=False)
                        for dt in range(KT):
                            lhsT = ftT_all[:, dt * N + q * P: dt * N + (q + 1) * P]
                            for c in range(NCH):
                                nc.tensor.matmul(
                                    G[:, c * 512:(c + 1) * 512], lhsT,
                                    ftT_all[:, dt * N + c * 512: dt * N + (c + 1) * 512],
                                    start=False, stop=(dt == KT - 1))
                        negM = stats.tile([P, 1], f32, tag="negM")
                        nc.vector.reduce_max(negM, G, axis=mybir.AxisListType.X,
                                             negate=True)
                        E = attn.tile([P, D], bf16, tag="E")
                        sums = stats.tile([P, 1], f32, tag="sums")
                        nc.scalar.activation(E, G, mybir.ActivationFunctionType.Exp,
                                             bias=negM, accum_out=sums)
                        recip = stats.tile([P, 1], f32, tag="recip")
                        nc.vector.reciprocal(recip, sums)
                        Es[q] = E
                        recips[q] = recip
                    if it > 0:
                        p = it - 1
                        E_p = Es[p]
                        # E^T via PE transposes, staged in the PV psum slot
                        stag = pv_ps.tile([P, D], bf16, tag="pv")
                        for mt in range(NT):
                            nc.tensor.transpose(stag[:, mt * P:(mt + 1) * P],
                                                E_p[:, mt * P:(mt + 1) * P], id_bf16)
                        ET = et_pool.tile([P, D], f32r, tag="ET")
                        for g in range(NCH):
                            nc.vector.tensor_copy(ET[:, g * 512:(g + 1) * 512],
                                                  stag[:, g * 512:(g + 1) * 512])
                        # PV: out[p-block, :] = E_p @ ft   (accumulate over m tiles)
                        pv = pv_ps.tile([P, D], f32, tag="pv")
                        for mt in range(NT):
                            lhsT = ET[:, mt * P:(mt + 1) * P]
                            for c in range(NCH):
                                nc.tensor.matmul(
                                    pv[:, c * 512:(c + 1) * 512], lhsT,
                                    ft_all[:, mt * D + c * 512: mt * D + (c + 1) * 512],
                                    start=(mt == 0), stop=(mt == NT - 1))
                        osb = attn.tile([P, D], f32, tag="osb")
                        nc.scalar.activation(osb, pv,
                                             mybir.ActivationFunctionType.Copy,
                                             scale=recips[p])
                        nc.sync.dma_start(out=out_d.ap()[p * P:(p + 1) * P, :], in_=osb)

    nc.compile()
    return nc


def _get_nc(reps=1):
    if reps not in _built:
        _built[reps] = _build(reps)
    return _built[reps]


def kernel(features, adj=None, attn_weights=None, attn_bias=None, _trace=False,
           _reps=1, **_ignored):
    nc = _get_nc(_reps)
    features = np.ascontiguousarray(np.asarray(features, dtype=np.float32))
    W = np.ascontiguousarray(np.asarray(attn_weights, dtype=np.float32))
    b = np.ascontiguousarray(np.asarray(attn_bias, dtype=np.float32)).reshape(1, D)
    in_maps = [
        {"features": features[i], "attn_weights": W, "attn_bias": b}
        for i in range(B)
    ]
    res = run_bass_kernel_spmd(nc, in_maps, list(range(B)), trace=_trace)
    out = np.stack([res.results[i]["out"] for i in range(B)], axis=0)
    if _trace:
        return out, res
    return out
